# revision 1
# baseline (speedup 1.0000x reference)
"""ComplEx scoring kernel for 8 Trainium2 NeuronCores.

Math: score[b, e] = Re(<h_b * r_b, conj(ent_e)>) with h = ent_emb[triples[:,0]],
r = rel_emb[triples[:,1]].  Writing ans_b = concat(re_h*re_r - im_h*im_r,
re_h*im_r + im_h*re_r) (shape [B, 512]), the score is exactly
score = ans @ ent_emb.T  — one [1024, 512] x [512, 200000] GEMM.

Strategy (vocab/tensor parallel on the entity axis, 25000 entities/core,
padded to 25088 = 49x512 columns):
  - host: tiny gather + complex multiply -> ans  (microseconds)
  - the GEMM is TensorE-bound (26.3 GFLOP/core vs 78.6 TF/s bf16), so the
    entity axis is split into a bf16 part (36 tiles of 512) and an fp8-e4m3
    DoubleRow part (13 tiles) that runs the PE at 2 MACs/cell/cycle.  The
    fp8 fraction (26% of entities) is sized so the fp8 quantization noise
    (3.75e-2 on those columns) keeps the global rel err at ~1.93e-2 (<2e-2).
  - per core: score_bf16[1024, 18432] + score_fp8[1024, 6656], both f16.
    PE is pre-warmed with dummy matmuls so the HAM clock gate opens during
    the preamble/first DMAs instead of 3.4us into real work.  Inputs ride
    the ACT HWDGE ring, outputs the SP ring.  The fp8 section runs second
    to last: its score writes come at 2x the bf16 rate (~300 GB/s), and the
    small final bf16 group absorbs the write backlog so the kernel tail is
    just the last small DMA's completion receipt.
  - host: concatenate the 8 column slabs, per-region unscale, drop padding
"""

import numpy as np
import ml_dtypes

NCORES = 8
NUM_ENT = 200000
EMB = 512
B = 1024
SHARD = NUM_ENT // NCORES      # 25000 entities per core
NTILE = 512                    # matmul moving free dim == one PSUM bank
NB_TILES = 36                  # bf16 512-tiles per core
NF_TILES = 13                  # fp8 512-tiles per core
NB = NB_TILES * NTILE          # 18432 bf16 columns
NF = NF_TILES * NTILE          # 6656 fp8 columns (6568 real + 88 pad)
SHARD_PAD = NB + NF            # 25088
NF_REAL = SHARD - NB           # 6568 real entities in the fp8 region
GROUPS = [4, 4, 7, 7, 7, 5, 2] # bf16 tile groups (DMA/reuse granularity)
GN_FULL = 7 * NTILE            # 3584
KCH = EMB // 128               # 4 contraction chunks
MCH = B // 128                 # 8 batch chunks

_NC = None
_SCALES = {}

# bf16-path score values are ~1e-5 — subnormal in fp16.  Pre-scaling ans by
# 2**16 on the host puts the device-side scores in fp16's normal range; the
# host unscales.  The fp8 path has its own scales (s_a, s_e) chosen at prep
# time so quantized inputs sit in e4m3's range and scores fit fp16.
OUT_SCALE = 2.0 ** 16


def _build_nc():
    import concourse.bacc as bacc
    import concourse.bass as bass
    import concourse.tile as tile
    from concourse import mybir

    ts, ds = bass.ts, bass.ds
    bf16 = mybir.dt.bfloat16
    f16 = mybir.dt.float16
    f8 = mybir.dt.float8e4
    f32 = mybir.dt.float32
    DR = mybir.MatmulPerfMode.DoubleRow

    nc = bacc.Bacc("TRN2", target_bir_lowering=False, debug=False)
    ansT = nc.dram_tensor("ansT", [EMB, B], bf16, kind="ExternalInput")
    ans8 = nc.dram_tensor("ans8", [128, KCH, B], f8, kind="ExternalInput")
    entT = nc.dram_tensor("entT", [EMB, NB], bf16, kind="ExternalInput")
    ent8 = nc.dram_tensor("ent8", [128, KCH, NF], f8, kind="ExternalInput")
    score = nc.dram_tensor("score", [B, SHARD_PAD], f16, kind="ExternalOutput")

    with tile.TileContext(nc) as tc:
        with tc.tile_pool(name="entp", bufs=3 * KCH) as ent_pool, \
             tc.tile_pool(name="outp", bufs=4) as out_pool, \
             tc.tile_pool(name="out8p", bufs=3) as out8_pool, \
             tc.tile_pool(name="ps", bufs=8, space="PSUM") as psum_pool:

            _frees = []
            ansT_sb, _f = tc.tile([128, KCH, B], bf16, name="ansT_sb")
            _frees.append(_f)
            ans8_sb, _f = tc.tile([128, KCH, B], f8, name="ans8_sb")
            _frees.append(_f)
            ent8_sb, _f = tc.tile([128, KCH, NF], f8, name="ent8_sb")
            _frees.append(_f)
            wup, _f = tc.tile([128, 640], bf16, name="wup")
            _frees.append(_f)

            # PE pre-warm: ~4us of zero matmuls so the HAM clock gate opens
            # during the preamble/DMA wait; real matmuls then run at 2.4 GHz
            # from the first instruction.
            nc.gpsimd.memset(wup[:], 0)
            wps = psum_pool.tile([128, NTILE], f32, name="pst")
            for i in range(12):
                nc.tensor.matmul(wps[:], wup[:, ds(0, 128)],
                                 wup[:, ds(128, 512)],
                                 start=(i == 0), stop=(i == 11))

            # inputs ride the ACT HWDGE ring (nc.scalar) — it comes out of
            # the preamble ~2.5us before the SP ring and keeps prefetches
            # from queueing behind score-output DMAs; outputs ride nc.sync
            def load_group(g, gcol):
                # one tile per k-chunk so a matmul only waits for its own DMA
                gn = GROUPS[g] * NTILE
                tiles = []
                for k in range(KCH):
                    t = ent_pool.tile([128, GN_FULL], bf16, name="ent_sb")
                    nc.scalar.dma_start(t[:, ds(0, gn)],
                                        entT[ts(k, 128), ds(gcol, gn)])
                    tiles.append(t)
                return tiles

            # startup: dma_start issue costs ~650ns of sequencer time apiece,
            # so use few, large DMAs.  k-slab order matches the first block's
            # k-outer consume order: the first matmul waits only for
            # ansT[k0] + the k0 ent slab (~0.8 MB).
            ent_sb0 = [ent_pool.tile([128, GN_FULL], bf16, name="ent_sb")
                       for _ in range(KCH)]
            gn0 = GROUPS[0] * NTILE
            nc.scalar.dma_start(ansT_sb[:, 0], ansT[ts(0, 128), :])
            nc.scalar.dma_start(ent_sb0[0][:, ds(0, gn0)],
                                entT[ts(0, 128), ds(0, gn0)])
            for k in range(1, KCH):
                nc.scalar.dma_start(ansT_sb[:, k], ansT[ts(k, 128), :])
                nc.scalar.dma_start(ent_sb0[k][:, ds(0, gn0)],
                                    entT[ts(k, 128), ds(0, gn0)])

            # gpsimd (Pool) cannot read PSUM on TRN2 — copyback on DVE + Act
            copy_engines = [nc.vector, nc.scalar]
            ci = 0

            def copyback(dst, ps):
                nonlocal ci
                eng = copy_engines[ci % len(copy_engines)]
                ci += 1
                if eng is nc.scalar:
                    eng.copy(dst, ps)
                else:
                    eng.tensor_copy(out=dst, in_=ps)

            ent_tiles = {0: ent_sb0}
            gcols = np.cumsum([0] + [gs * NTILE for gs in GROUPS]).tolist()

            def load_fp8():
                nc.scalar.dma_start(ans8_sb[:], ans8[:, :, :])
                nc.scalar.dma_start(ent8_sb[:, ds(0, 2)], ent8[:, ds(0, 2), :])
                nc.scalar.dma_start(ent8_sb[:, ds(2, 2)], ent8[:, ds(2, 2), :])

            def bf16_group(g, warm=False, last=False):
                gsz = GROUPS[g]
                gn = gsz * NTILE
                col = gcols[g]
                ent_sb = ent_tiles.pop(g)

                if warm:
                    # warm-up: k-outer with m0+m1 interleaved (2*gsz = 8 psum
                    # banks) so each k ent slab feeds 8 matmuls (~1.8us) —
                    # faster than the ~1.5us the next slab's DMA takes, so
                    # the PE never starves while group 0 lands
                    outs = [out_pool.tile([128, GN_FULL], f16, name="out_sb")
                            for _ in range(2)]
                    pss0 = [[psum_pool.tile([128, NTILE], f32, name="pst")
                             for _ in range(gsz)] for _ in range(2)]
                    for k in range(KCH):
                        for m in range(2):
                            lhsT = ansT_sb[:, k, ts(m, 128)]
                            for t in range(gsz):
                                nc.tensor.matmul(
                                    pss0[m][t][:], lhsT,
                                    ent_sb[k][:, ts(t, NTILE)],
                                    start=(k == 0), stop=(k == KCH - 1))
                                if k == KCH - 1:
                                    copyback(outs[m][:, ts(t, NTILE)],
                                             pss0[m][t][:])
                    h0 = (gsz // 2) * NTILE
                    for m in range(2):
                        nc.sync.dma_start(score[ts(m, 128), ds(col, h0)],
                                          outs[m][:, ds(0, h0)])
                        nc.sync.dma_start(score[ts(m, 128), ds(col + h0, gn - h0)],
                                          outs[m][:, ds(h0, gn - h0)])
                    ms = range(2, MCH)
                else:
                    ms = range(MCH)

                for m in ms:
                    pss = [psum_pool.tile([128, NTILE], f32, name="pst")
                           for _ in range(gsz)]
                    out_sb = out_pool.tile([128, GN_FULL], f16, name="out_sb")
                    # k outer: keeps the PE streaming one ent tile after
                    # another with the same weight chunk
                    for k in range(KCH):
                        lhsT = ansT_sb[:, k, ts(m, 128)]
                        for t in range(gsz):
                            nc.tensor.matmul(
                                pss[t][:], lhsT, ent_sb[k][:, ts(t, NTILE)],
                                start=(k == 0), stop=(k == KCH - 1))
                    for t in range(gsz):
                        copyback(out_sb[:, ts(t, NTILE)], pss[t][:])
                    if last and m == MCH - 1:
                        # fine-grained final drain: the kernel's tail is the
                        # last DMA's completion receipt, keep it small
                        for t in range(gsz - 1):
                            nc.sync.dma_start(
                                score[ts(m, 128), ds(col + t * NTILE, NTILE)],
                                out_sb[:, ds(t * NTILE, NTILE)])
                        base = (gsz - 1) * NTILE
                        nc.sync.dma_start(score[ts(m, 128), ds(col + base, 256)],
                                          out_sb[:, ds(base, 256)])
                        nc.sync.dma_start(
                            score[ts(m, 128), ds(col + base + 256, 256)],
                            out_sb[:, ds(base + 256, 256)])
                    elif gsz >= 4:
                        # two half-width output DMAs so the drain starts as
                        # soon as the first copies land
                        h0 = (gsz // 2 + 1) * NTILE
                        nc.sync.dma_start(score[ts(m, 128), ds(col, h0)],
                                          out_sb[:, ds(0, h0)])
                        nc.sync.dma_start(
                            score[ts(m, 128), ds(col + h0, gn - h0)],
                            out_sb[:, ds(h0, gn - h0)])
                    else:
                        nc.sync.dma_start(score[ts(m, 128), ds(col, gn)],
                                          out_sb[:, ds(0, gn)])

            def fp8_section():
                # fp8 DoubleRow: K=512 as 2 matmuls of 256 (2 fp8/cell).
                # Runs mid-kernel: its score writes come at 2x the bf16 rate
                # (~300 GB/s), so the surrounding bf16 groups' write slack
                # absorbs the burst instead of stretching the kernel tail.
                col8 = gcols[-1]
                for m in range(MCH):
                    out_sb = out8_pool.tile([128, NF], f16, name="out8_sb")
                    for t in range(NF_TILES):
                        ps = psum_pool.tile([128, NTILE], f32, name="pst")
                        for j in range(2):
                            nc.tensor.matmul(
                                ps[:],
                                ans8_sb[:, ds(2 * j, 2), ts(m, 128)],
                                ent8_sb[:, ds(2 * j, 2), ds(t * NTILE, NTILE)],
                                start=(j == 0), stop=(j == 1),
                                perf_mode=DR)
                        copyback(out_sb[:, ts(t, NTILE)], ps[:])
                    h0 = 6 * NTILE
                    nc.sync.dma_start(score[ts(m, 128), ds(col8, h0)],
                                      out_sb[:, ds(0, h0)])
                    nc.sync.dma_start(score[ts(m, 128), ds(col8 + h0, NF - h0)],
                                      out_sb[:, ds(h0, NF - h0)])

            # process order: b0..b5, fp8, b6 — each section's inputs are
            # issued one section ahead on the ACT ring.  fp8 sits second to
            # last: during its span only b6's small load (1 MB) competes with
            # its 2x-rate score writes (325 GB/s total, under the 358 limit),
            # and the b6 epilogue absorbs the write backlog so the kernel
            # tail stays short.
            ent_tiles[1] = load_group(1, gcols[1])
            bf16_group(0, warm=True)
            ent_tiles[2] = load_group(2, gcols[2])
            bf16_group(1)
            ent_tiles[3] = load_group(3, gcols[3])
            bf16_group(2)
            ent_tiles[4] = load_group(4, gcols[4])
            bf16_group(3)
            ent_tiles[5] = load_group(5, gcols[5])
            bf16_group(4)
            load_fp8()
            bf16_group(5)
            ent_tiles[6] = load_group(6, gcols[6])
            fp8_section()
            bf16_group(6, last=True)
            for _f in reversed(_frees):
                _f()
    nc.compile()
    return nc


def _get_nc():
    global _NC
    if _NC is None:
        _NC = _build_nc()
    return _NC


def _pmap(fn, n):
    from concurrent.futures import ThreadPoolExecutor
    with ThreadPoolExecutor(max_workers=n) as ex:
        list(ex.map(fn, range(n)))


def _to_f8_chunks(mat_t, ncols):
    """[EMB, ncols] f32 (already scaled) -> [128, KCH, ncols] e4m3 bytes."""
    q = mat_t.astype(ml_dtypes.float8_e4m3fn)
    return np.ascontiguousarray(q.reshape(KCH, 128, ncols).transpose(1, 0, 2))


def prepare_in_maps(triples, ent_emb, rel_emb):
    triples = np.asarray(triples)
    ent_emb = np.asarray(ent_emb, dtype=np.float32)
    rel_emb = np.asarray(rel_emb, dtype=np.float32)

    d = EMB // 2
    h = ent_emb[triples[:, 0].astype(np.int64)]
    r = rel_emb[triples[:, 1].astype(np.int64)]
    re_h, im_h = h[:, :d], h[:, d:]
    re_r, im_r = r[:, :d], r[:, d:]
    ans = np.empty((B, EMB), np.float32)
    ans[:, :d] = re_h * re_r - im_h * im_r
    ans[:, d:] = re_h * im_r + im_h * re_r

    ansT_bf = np.ascontiguousarray(ans.T * np.float32(OUT_SCALE)).astype(
        ml_dtypes.bfloat16)

    # fp8 scales: map absmax to ~120 (TRN e4m3 max 240), then cap the product
    # so the Cauchy-Schwarz bound on device-side scores stays inside fp16
    f8_rows = np.concatenate([
        ent_emb[c * SHARD + NB:(c + 1) * SHARD] for c in range(NCORES)])
    amax_a = float(np.abs(ans).max())
    amax_e = float(np.abs(f8_rows).max())
    s_a = 120.0 / amax_a
    s_e = 120.0 / amax_e
    cs = float(np.sqrt((ans * ans).sum(1).max()) *
               np.sqrt((f8_rows * f8_rows).sum(1).max()))
    cap = 58000.0 / cs
    if s_a * s_e > cap:
        s_a = cap / s_e
    _SCALES["fp8_inv"] = 1.0 / (s_a * s_e)

    ans8 = _to_f8_chunks(np.ascontiguousarray(ans.T) * np.float32(s_a), B)

    ent_bf = np.empty((NCORES, EMB, NB), dtype=ml_dtypes.bfloat16)
    ent8s = np.empty((NCORES, 128, KCH, NF), dtype=ml_dtypes.float8_e4m3fn)

    def _core(c):
        rows = ent_emb[c * SHARD:(c + 1) * SHARD]
        ent_bf[c] = rows[:NB].T
        blk = np.zeros((EMB, NF), np.float32)
        blk[:, :NF_REAL] = rows[NB:].T * np.float32(s_e)
        ent8s[c] = _to_f8_chunks(blk, NF)

    _pmap(_core, NCORES)
    return [{"ansT": ansT_bf, "ans8": ans8, "entT": ent_bf[c],
             "ent8": ent8s[c]} for c in range(NCORES)]


def run_raw(in_maps, trace=False):
    from concourse import bass_utils
    return bass_utils.run_bass_kernel_spmd(
        _get_nc(), in_maps, core_ids=list(range(NCORES)), trace=trace
    )


def assemble(results):
    out = np.empty((B, NUM_ENT), np.float32)
    inv16 = np.float32(1.0 / OUT_SCALE)
    inv8 = np.float32(_SCALES["fp8_inv"])

    def _one(c):
        sh = results[c]["score"]
        bf = sh[:, :NB].astype(np.float32)
        bf *= inv16
        f8 = sh[:, NB:NB + NF_REAL].astype(np.float32)
        f8 *= inv8
        out[:, c * SHARD:c * SHARD + NB] = bf
        out[:, c * SHARD + NB:(c + 1) * SHARD] = f8

    _pmap(_one, NCORES)
    return out


def kernel(triples, ent_emb, rel_emb):
    in_maps = prepare_in_maps(triples, ent_emb, rel_emb)
    res = run_raw(in_maps)
    return assemble(res.results)



# revision 2
# speedup vs baseline: 1.0030x; 1.0030x over previous
"""ComplEx scoring kernel for 8 Trainium2 NeuronCores.

Math: score[b, e] = Re(<h_b * r_b, conj(ent_e)>) with h = ent_emb[triples[:,0]],
r = rel_emb[triples[:,1]].  Writing ans_b = concat(re_h*re_r - im_h*im_r,
re_h*im_r + im_h*re_r) (shape [B, 512]), the score is exactly
score = ans @ ent_emb.T  — one [1024, 512] x [512, 200000] GEMM.

Strategy (vocab/tensor parallel on the entity axis, 25000 entities/core,
padded to 25088 = 49x512 columns):
  - host: tiny gather + complex multiply -> ans  (microseconds)
  - the GEMM is TensorE-bound (26.3 GFLOP/core vs 78.6 TF/s bf16), so the
    entity axis is split into a bf16 part (34.5 tiles of 512) and an
    fp8-e4m3 DoubleRow part (14.5 tiles) running the PE at 2 MACs/cell/cy.
  - fp8 noise is relative per column, so each column's squared error is
    proportional to its score energy ~ ||ent_row||^2.  Assigning the
    smallest-norm entities of each shard to the fp8 region (instead of an
    arbitrary 26%) cuts the fp8 error ~8%, which funds a 29.3% fp8
    fraction at the same global rel err (~1.96e-2 < 2e-2).  The host
    unscrambles the column permutation during assemble.
  - per core: score_bf16[1024, 17664] + score_fp8[1024, 7424], both f16.
    PE is pre-warmed with dummy matmuls so the HAM clock gate opens during
    the preamble/first DMAs.  The first two input DMAs are split across
    the SP and ACT HWDGE rings so the first slab lands ~1us earlier.
    The fp8 section runs second to last; a final 256-column bf16 group
    absorbs its write backlog so the kernel tail is one small DMA.
  - host: per-region unscale + column scatter back to entity order.
"""

import numpy as np
import ml_dtypes

NCORES = 8
NUM_ENT = 200000
EMB = 512
B = 1024
SHARD = NUM_ENT // NCORES      # 25000 entities per core
NTILE = 512                    # matmul moving free dim == one PSUM bank
NB_FULL = 34                   # full bf16 512-tiles per core
NB_PART = 256                  # trailing partial bf16 tile width
NB = NB_FULL * NTILE + NB_PART # 17664 bf16 columns (all real)
NF_FULL = 14                   # full fp8 512-tiles per core
NF_PART = 256                  # trailing partial fp8 tile width
NF = NF_FULL * NTILE + NF_PART # 7424 fp8 columns (7336 real + 88 pad)
SHARD_PAD = NB + NF            # 25088
NF_REAL = SHARD - NB           # 7336 real entities in the fp8 region
GROUPS = [4, 4, 7, 7, 7, 5]    # full bf16 tile groups (DMA/reuse granularity)
GN_FULL = 7 * NTILE            # 3584
KCH = EMB // 128               # 4 contraction chunks
MCH = B // 128                 # 8 batch chunks
WARMUP_MMS = 9

_NC = None
_SCALES = {}

# bf16-path score values are ~1e-5 — subnormal in fp16.  Pre-scaling ans by
# 2**16 on the host puts the device-side scores in fp16's normal range; the
# host unscales.  The fp8 path has its own scales (s_a, s_e) chosen at prep
# time so quantized inputs sit in e4m3's range and scores fit fp16.
OUT_SCALE = 2.0 ** 16


def _build_nc():
    import concourse.bacc as bacc
    import concourse.bass as bass
    import concourse.tile as tile
    from concourse import mybir

    ts, ds = bass.ts, bass.ds
    bf16 = mybir.dt.bfloat16
    f16 = mybir.dt.float16
    f8 = mybir.dt.float8e4
    f32 = mybir.dt.float32
    DR = mybir.MatmulPerfMode.DoubleRow

    nc = bacc.Bacc("TRN2", target_bir_lowering=False, debug=False)
    ansT = nc.dram_tensor("ansT", [EMB, B], bf16, kind="ExternalInput")
    ans8 = nc.dram_tensor("ans8", [128, KCH, B], f8, kind="ExternalInput")
    entT = nc.dram_tensor("entT", [EMB, NB_FULL * NTILE], bf16,
                          kind="ExternalInput")
    entP = nc.dram_tensor("entP", [128, KCH, NB_PART], bf16,
                          kind="ExternalInput")
    ent8 = nc.dram_tensor("ent8", [128, KCH, NF], f8, kind="ExternalInput")
    score = nc.dram_tensor("score", [B, SHARD_PAD], f16, kind="ExternalOutput")

    with tile.TileContext(nc) as tc:
        with tc.tile_pool(name="entp", bufs=3 * KCH) as ent_pool, \
             tc.tile_pool(name="outp", bufs=4) as out_pool, \
             tc.tile_pool(name="out8p", bufs=2) as out8_pool, \
             tc.tile_pool(name="ps", bufs=8, space="PSUM") as psum_pool:

            _frees = []
            ansT_sb, _f = tc.tile([128, KCH, B], bf16, name="ansT_sb")
            _frees.append(_f)
            ans8_sb, _f = tc.tile([128, KCH, B], f8, name="ans8_sb")
            _frees.append(_f)
            entP_sb, _f = tc.tile([128, KCH, NB_PART], bf16, name="entP_sb")
            _frees.append(_f)
            ent8_sb, _f = tc.tile([128, KCH, NF], f8, name="ent8_sb")
            _frees.append(_f)
            wup, _f = tc.tile([128, 640], bf16, name="wup")
            _frees.append(_f)

            # PE pre-warm: zero matmuls so the HAM clock gate opens during
            # the preamble/DMA wait; real matmuls then run at 2.4 GHz.
            nc.gpsimd.memset(wup[:], 0)
            wps = psum_pool.tile([128, NTILE], f32, name="pst")
            for i in range(WARMUP_MMS):
                nc.tensor.matmul(wps[:], wup[:, ds(0, 128)],
                                 wup[:, ds(128, 512)],
                                 start=(i == 0), stop=(i == WARMUP_MMS - 1))

            # startup: the first real matmul needs ansT[k0] + the first two
            # ent tiles of group 0.  Those two DMAs are issued in parallel,
            # one on the SP ring (idle until the first score write) and one
            # on the ACT ring, so the dependency lands ~1us earlier than a
            # serial issue on one ring.
            ent_sb0 = [ent_pool.tile([128, GN_FULL], bf16, name="ent_sb")
                       for _ in range(KCH)]
            gn0 = GROUPS[0] * NTILE
            nc.sync.dma_start(ansT_sb[:, 0], ansT[ts(0, 128), :])
            nc.scalar.dma_start(ent_sb0[0][:, ds(0, 1024)],
                                entT[ts(0, 128), ds(0, 1024)])
            nc.scalar.dma_start(ent_sb0[0][:, ds(1024, gn0 - 1024)],
                                entT[ts(0, 128), ds(1024, gn0 - 1024)])
            for k in range(1, KCH):
                nc.scalar.dma_start(ansT_sb[:, k], ansT[ts(k, 128), :])
                nc.scalar.dma_start(ent_sb0[k][:, ds(0, gn0)],
                                    entT[ts(k, 128), ds(0, gn0)])

            # inputs ride the ACT HWDGE ring (nc.scalar) — it keeps
            # prefetches from queueing behind score-output DMAs on SP.
            def load_group(g, gcol):
                # one tile per k-chunk so a matmul only waits for its own DMA
                gn = GROUPS[g] * NTILE
                tiles = []
                for k in range(KCH):
                    t = ent_pool.tile([128, GN_FULL], bf16, name="ent_sb")
                    nc.scalar.dma_start(t[:, ds(0, gn)],
                                        entT[ts(k, 128), ds(gcol, gn)])
                    tiles.append(t)
                return tiles

            # gpsimd (Pool) cannot read PSUM on TRN2 — copyback on DVE + Act
            copy_engines = [nc.vector, nc.scalar]
            ci = 0

            def copyback(dst, ps):
                nonlocal ci
                eng = copy_engines[ci % len(copy_engines)]
                ci += 1
                if eng is nc.scalar:
                    eng.copy(dst, ps)
                else:
                    eng.tensor_copy(out=dst, in_=ps)

            ent_tiles = {0: ent_sb0}
            gcols = np.cumsum([0] + [gs * NTILE for gs in GROUPS]).tolist()
            COLP = gcols[-1]               # partial bf16 tile column base
            COL8 = NB                      # fp8 region column base

            def load_fp8():
                nc.scalar.dma_start(ans8_sb[:], ans8[:, :, :])
                nc.scalar.dma_start(ent8_sb[:, ds(0, 2)], ent8[:, ds(0, 2), :])
                nc.scalar.dma_start(ent8_sb[:, ds(2, 2)], ent8[:, ds(2, 2), :])

            def load_part():
                nc.scalar.dma_start(entP_sb[:], entP[:, :, :])

            def bf16_group(g, warm=False):
                gsz = GROUPS[g]
                gn = gsz * NTILE
                col = gcols[g]
                ent_sb = ent_tiles.pop(g)

                if warm:
                    # warm-up: k-outer with m0+m1 interleaved (2*gsz = 8 psum
                    # banks) so each k ent slab feeds 8 matmuls (~1.8us) —
                    # faster than the next slab's DMA, so the PE never
                    # starves while group 0 lands
                    outs = [out_pool.tile([128, GN_FULL], f16, name="out_sb")
                            for _ in range(2)]
                    pss0 = [[psum_pool.tile([128, NTILE], f32, name="pst")
                             for _ in range(gsz)] for _ in range(2)]
                    for k in range(KCH):
                        for m in range(2):
                            lhsT = ansT_sb[:, k, ts(m, 128)]
                            for t in range(gsz):
                                nc.tensor.matmul(
                                    pss0[m][t][:], lhsT,
                                    ent_sb[k][:, ts(t, NTILE)],
                                    start=(k == 0), stop=(k == KCH - 1))
                                if k == KCH - 1:
                                    copyback(outs[m][:, ts(t, NTILE)],
                                             pss0[m][t][:])
                    h0 = (gsz // 2) * NTILE
                    for m in range(2):
                        nc.sync.dma_start(score[ts(m, 128), ds(col, h0)],
                                          outs[m][:, ds(0, h0)])
                        nc.sync.dma_start(score[ts(m, 128), ds(col + h0, gn - h0)],
                                          outs[m][:, ds(h0, gn - h0)])
                    ms = range(2, MCH)
                else:
                    ms = range(MCH)

                for m in ms:
                    pss = [psum_pool.tile([128, NTILE], f32, name="pst")
                           for _ in range(gsz)]
                    out_sb = out_pool.tile([128, GN_FULL], f16, name="out_sb")
                    # k outer: keeps the PE streaming one ent tile after
                    # another with the same weight chunk
                    for k in range(KCH):
                        lhsT = ansT_sb[:, k, ts(m, 128)]
                        for t in range(gsz):
                            nc.tensor.matmul(
                                pss[t][:], lhsT, ent_sb[k][:, ts(t, NTILE)],
                                start=(k == 0), stop=(k == KCH - 1))
                    for t in range(gsz):
                        copyback(out_sb[:, ts(t, NTILE)], pss[t][:])
                    if gsz >= 4:
                        # two half-width output DMAs so the drain starts as
                        # soon as the first copies land
                        h0 = (gsz // 2 + 1) * NTILE
                        nc.sync.dma_start(score[ts(m, 128), ds(col, h0)],
                                          out_sb[:, ds(0, h0)])
                        nc.sync.dma_start(
                            score[ts(m, 128), ds(col + h0, gn - h0)],
                            out_sb[:, ds(h0, gn - h0)])
                    else:
                        nc.sync.dma_start(score[ts(m, 128), ds(col, gn)],
                                          out_sb[:, ds(0, gn)])

            def fp8_section():
                # fp8 DoubleRow: K=512 as 2 matmuls of 256 (2 fp8/cell).
                # Runs second to last: its score writes come at 2x the bf16
                # rate (~300 GB/s); the SP ring absorbs them at ~340 GB/s
                # once the input loads are done, and the final small bf16
                # group drains the residue.
                for m in range(MCH):
                    out_sb = out8_pool.tile([128, NF], f16, name="out8_sb")
                    for t in range(NF_FULL):
                        ps = psum_pool.tile([128, NTILE], f32, name="pst")
                        for j in range(2):
                            nc.tensor.matmul(
                                ps[:],
                                ans8_sb[:, ds(2 * j, 2), ts(m, 128)],
                                ent8_sb[:, ds(2 * j, 2), ds(t * NTILE, NTILE)],
                                start=(j == 0), stop=(j == 1),
                                perf_mode=DR)
                        copyback(out_sb[:, ts(t, NTILE)], ps[:])
                    # trailing partial fp8 tile
                    ps = psum_pool.tile([128, NTILE], f32, name="pst")
                    for j in range(2):
                        nc.tensor.matmul(
                            ps[:, ds(0, NF_PART)],
                            ans8_sb[:, ds(2 * j, 2), ts(m, 128)],
                            ent8_sb[:, ds(2 * j, 2),
                                    ds(NF_FULL * NTILE, NF_PART)],
                            start=(j == 0), stop=(j == 1),
                            perf_mode=DR)
                    copyback(out_sb[:, ds(NF_FULL * NTILE, NF_PART)],
                             ps[:, ds(0, NF_PART)])
                    h0 = 7 * NTILE
                    nc.sync.dma_start(score[ts(m, 128), ds(COL8, h0)],
                                      out_sb[:, ds(0, h0)])
                    nc.sync.dma_start(score[ts(m, 128), ds(COL8 + h0, NF - h0)],
                                      out_sb[:, ds(h0, NF - h0)])

            def part_group():
                # final 256-column bf16 group: small PE steps whose tiny
                # output DMAs drain behind the fp8 burst, keeping the
                # kernel tail to one 64KB write + its receipt.
                for m in range(MCH):
                    ps = psum_pool.tile([128, NTILE], f32, name="pst")
                    out_sb = out_pool.tile([128, GN_FULL], f16, name="out_sb")
                    for k in range(KCH):
                        nc.tensor.matmul(
                            ps[:, ds(0, NB_PART)],
                            ansT_sb[:, k, ts(m, 128)],
                            entP_sb[:, k, :],
                            start=(k == 0), stop=(k == KCH - 1))
                    copyback(out_sb[:, ds(0, NB_PART)], ps[:, ds(0, NB_PART)])
                    nc.sync.dma_start(score[ts(m, 128), ds(COLP, NB_PART)],
                                      out_sb[:, ds(0, NB_PART)])

            # process order: b0..b5, fp8, bpart — each section's inputs are
            # issued at least one section ahead on the ACT ring.
            ent_tiles[1] = load_group(1, gcols[1])
            bf16_group(0, warm=True)
            ent_tiles[2] = load_group(2, gcols[2])
            bf16_group(1)
            ent_tiles[3] = load_group(3, gcols[3])
            bf16_group(2)
            ent_tiles[4] = load_group(4, gcols[4])
            bf16_group(3)
            load_fp8()
            bf16_group(4)
            ent_tiles[5] = load_group(5, gcols[5])
            load_part()
            bf16_group(5)
            fp8_section()
            part_group()
            for _f in reversed(_frees):
                _f()
    nc.compile()
    return nc


def _get_nc():
    global _NC
    if _NC is None:
        _NC = _build_nc()
    return _NC


def _pmap(fn, n):
    from concurrent.futures import ThreadPoolExecutor
    with ThreadPoolExecutor(max_workers=n) as ex:
        list(ex.map(fn, range(n)))


def _to_f8_chunks(mat_t, ncols):
    """[EMB, ncols] f32 (already scaled) -> [128, KCH, ncols] e4m3 bytes."""
    q = mat_t.astype(ml_dtypes.float8_e4m3fn)
    return np.ascontiguousarray(q.reshape(KCH, 128, ncols).transpose(1, 0, 2))


def prepare_in_maps(triples, ent_emb, rel_emb):
    triples = np.asarray(triples)
    ent_emb = np.asarray(ent_emb, dtype=np.float32)
    rel_emb = np.asarray(rel_emb, dtype=np.float32)

    d = EMB // 2
    h = ent_emb[triples[:, 0].astype(np.int64)]
    r = rel_emb[triples[:, 1].astype(np.int64)]
    re_h, im_h = h[:, :d], h[:, d:]
    re_r, im_r = r[:, :d], r[:, d:]
    ans = np.empty((B, EMB), np.float32)
    ans[:, :d] = re_h * re_r - im_h * im_r
    ans[:, d:] = re_h * im_r + im_h * re_r

    ansT_bf = np.ascontiguousarray(ans.T * np.float32(OUT_SCALE)).astype(
        ml_dtypes.bfloat16)

    # fp8 noise per column scales with the column's score energy, i.e. with
    # ||ent_row||^2 — so the smallest-norm entities of each shard go to the
    # fp8 region.  The host scatters columns back during assemble.
    norms = (ent_emb * ent_emb).sum(1)
    bf_idx = np.empty((NCORES, NB), np.int64)
    f8_idx = np.empty((NCORES, NF_REAL), np.int64)
    for c in range(NCORES):
        sl = norms[c * SHARD:(c + 1) * SHARD]
        order = np.argpartition(sl, NF_REAL)
        f8_idx[c] = np.sort(order[:NF_REAL])
        bf_idx[c] = np.sort(order[NF_REAL:])
    _SCALES["bf_idx"] = bf_idx
    _SCALES["f8_idx"] = f8_idx

    # fp8 scales: map absmax to ~120 (TRN e4m3 max 240), then cap the product
    # so the Cauchy-Schwarz bound on device-side scores stays inside fp16
    f8_rows = np.concatenate([
        ent_emb[c * SHARD + f8_idx[c]] for c in range(NCORES)])
    amax_a = float(np.abs(ans).max())
    amax_e = float(np.abs(f8_rows).max())
    s_a = 120.0 / amax_a
    s_e = 120.0 / amax_e
    cs = float(np.sqrt((ans * ans).sum(1).max()) *
               np.sqrt((f8_rows * f8_rows).sum(1).max()))
    cap = 58000.0 / cs
    if s_a * s_e > cap:
        s_a = cap / s_e
    _SCALES["fp8_inv"] = 1.0 / (s_a * s_e)

    ans8 = _to_f8_chunks(np.ascontiguousarray(ans.T) * np.float32(s_a), B)

    ent_bf = np.empty((NCORES, EMB, NB_FULL * NTILE), dtype=ml_dtypes.bfloat16)
    ent_pp = np.empty((NCORES, 128, KCH, NB_PART), dtype=ml_dtypes.bfloat16)
    ent8s = np.empty((NCORES, 128, KCH, NF), dtype=ml_dtypes.float8_e4m3fn)

    def _core(c):
        rows = ent_emb[c * SHARD:(c + 1) * SHARD]
        bfT = rows[bf_idx[c]].T                      # [EMB, NB]
        ent_bf[c] = bfT[:, :NB_FULL * NTILE]
        pp = np.ascontiguousarray(bfT[:, NB_FULL * NTILE:])
        ent_pp[c] = pp.reshape(KCH, 128, NB_PART).transpose(1, 0, 2)
        blk = np.zeros((EMB, NF), np.float32)
        blk[:, :NF_REAL] = rows[f8_idx[c]].T * np.float32(s_e)
        ent8s[c] = _to_f8_chunks(blk, NF)

    _pmap(_core, NCORES)
    return [{"ansT": ansT_bf, "ans8": ans8, "entT": ent_bf[c],
             "entP": ent_pp[c], "ent8": ent8s[c]} for c in range(NCORES)]


def run_raw(in_maps, trace=False):
    from concourse import bass_utils
    return bass_utils.run_bass_kernel_spmd(
        _get_nc(), in_maps, core_ids=list(range(NCORES)), trace=trace
    )


def assemble(results):
    out = np.empty((B, NUM_ENT), np.float32)
    inv16 = np.float32(1.0 / OUT_SCALE)
    inv8 = np.float32(_SCALES["fp8_inv"])
    bf_idx = _SCALES["bf_idx"]
    f8_idx = _SCALES["f8_idx"]

    def _one(c):
        sh = results[c]["score"]
        bf = sh[:, :NB].astype(np.float32)
        bf *= inv16
        f8 = sh[:, NB:NB + NF_REAL].astype(np.float32)
        f8 *= inv8
        base = c * SHARD
        out[:, base + bf_idx[c]] = bf
        out[:, base + f8_idx[c]] = f8

    _pmap(_one, NCORES)
    return out


def kernel(triples, ent_emb, rel_emb):
    in_maps = prepare_in_maps(triples, ent_emb, rel_emb)
    res = run_raw(in_maps)
    return assemble(res.results)


# revision 9
# speedup vs baseline: 1.0250x; 1.0219x over previous
"""ComplEx scoring kernel for 8 Trainium2 NeuronCores.

Math: score[b, e] = Re(<h_b * r_b, conj(ent_e)>) with h = ent_emb[triples[:,0]],
r = rel_emb[triples[:,1]].  Writing ans_b = concat(re_h*re_r - im_h*im_r,
re_h*im_r + im_h*re_r) (shape [B, 512]), the score is exactly
score = ans @ ent_emb.T  — one [1024, 512] x [512, 200000] GEMM.

Strategy (vocab/tensor parallel on the entity axis, 25000 entities/core,
padded to 25088 = 49x512 columns):
  - host: tiny gather + complex multiply -> ans  (microseconds)
  - the GEMM is TensorE-bound (26.3 GFLOP/core vs 78.6 TF/s bf16), so the
    entity axis is split into a bf16 part (34.5 tiles of 512) and an
    fp8-e4m3 DoubleRow part (14.5 tiles) running the PE at 2 MACs/cell/cy.
  - fp8 noise is relative per column, so each column's squared error is
    proportional to its score energy ~ ||ent_row||^2.  Assigning the
    smallest-norm entities of each shard to the fp8 region (instead of an
    arbitrary 26%) cuts the fp8 error ~8%, which funds a 29.3% fp8
    fraction at the same global rel err (~1.96e-2 < 2e-2).  The host
    unscrambles the column permutation during assemble.
  - per core: score_bf16[1024, 17664] + score_fp8[1024, 7424], both f16.
    PE is pre-warmed with dummy matmuls so the HAM clock gate opens during
    the preamble/first DMAs.  The first two input DMAs are split across
    the SP and ACT HWDGE rings so the first slab lands ~1us earlier.
    The fp8 section runs second to last; a final 256-column bf16 group
    absorbs its write backlog so the kernel tail is one small DMA.
  - host: per-region unscale + column scatter back to entity order.
"""

import numpy as np
import ml_dtypes

NCORES = 8
NUM_ENT = 200000
EMB = 512
B = 1024
SHARD = NUM_ENT // NCORES      # 25000 entities per core
NTILE = 512                    # matmul moving free dim == one PSUM bank
NB_FULL = 34                   # full bf16 512-tiles per core
NB_PART = 256                  # trailing partial bf16 tile width
NB = NB_FULL * NTILE + NB_PART # 17664 bf16 columns (all real)
NF_FULL = 14                   # full fp8 512-tiles per core
NF_PART = 256                  # trailing partial fp8 tile width
NF = NF_FULL * NTILE + NF_PART # 7424 fp8 columns (7336 real + 88 pad)
SHARD_PAD = NB + NF            # 25088
NF_REAL = SHARD - NB           # 7336 real entities in the fp8 region
GROUPS = [2, 5, 7, 7, 7, 5]    # full bf16 tile groups (DMA/reuse granularity)
NGRP = 33 * NTILE              # columns covered by GROUPS
NFIN = NB - NGRP               # 768: final group = 1 full tile + partial
GN_FULL = 7 * NTILE            # 3584
KCH = EMB // 128               # 4 contraction chunks
MCH = B // 128                 # 8 batch chunks
WARMUP_MMS = 9

_NC = None
_SCALES = {}

# bf16-path score values are ~1e-5 — subnormal in fp16.  Pre-scaling ans by
# 2**16 on the host puts the device-side scores in fp16's normal range; the
# host unscales.  The fp8 path has its own scales (s_a, s_e) chosen at prep
# time so quantized inputs sit in e4m3's range and scores fit fp16.
OUT_SCALE = 2.0 ** 16


def _build_nc():
    import concourse.bacc as bacc
    import concourse.bass as bass
    import concourse.tile as tile
    from concourse import mybir

    ts, ds = bass.ts, bass.ds
    bf16 = mybir.dt.bfloat16
    f16 = mybir.dt.float16
    f8 = mybir.dt.float8e4
    f32 = mybir.dt.float32
    DR = mybir.MatmulPerfMode.DoubleRow

    nc = bacc.Bacc("TRN2", target_bir_lowering=False, debug=False)
    ansT = nc.dram_tensor("ansT", [EMB, B], bf16, kind="ExternalInput")
    ans8 = nc.dram_tensor("ans8", [128, KCH, B], f8, kind="ExternalInput")
    entT = nc.dram_tensor("entT", [EMB, NGRP], bf16, kind="ExternalInput")
    entP = nc.dram_tensor("entP", [128, KCH, NFIN], bf16,
                          kind="ExternalInput")
    ent8 = nc.dram_tensor("ent8", [128, KCH, NF], f8, kind="ExternalInput")
    score = nc.dram_tensor("score", [B, SHARD_PAD], f16, kind="ExternalOutput")

    with tile.TileContext(nc) as tc:
        with tc.tile_pool(name="entp", bufs=3 * KCH) as ent_pool, \
             tc.tile_pool(name="outp", bufs=4) as out_pool, \
             tc.tile_pool(name="out8p", bufs=2) as out8_pool, \
             tc.tile_pool(name="ps", bufs=8, space="PSUM") as psum_pool:

            _frees = []
            ansT_sb, _f = tc.tile([128, KCH, B], bf16, name="ansT_sb")
            _frees.append(_f)
            ans8_sb, _f = tc.tile([128, KCH, B], f8, name="ans8_sb")
            _frees.append(_f)
            entP_sb, _f = tc.tile([128, KCH, NFIN], bf16, name="entP_sb")
            _frees.append(_f)
            ent8_sb, _f = tc.tile([128, KCH, NF], f8, name="ent8_sb")
            _frees.append(_f)
            wup, _f = tc.tile([128, 640], bf16, name="wup")
            _frees.append(_f)

            # PE pre-warm: zero matmuls so the HAM clock gate opens during
            # the preamble/DMA wait; real matmuls then run at 2.4 GHz.
            nc.gpsimd.memset(wup[:], 0)
            wps = psum_pool.tile([128, NTILE], f32, name="pst")
            for i in range(WARMUP_MMS):
                nc.tensor.matmul(wps[:], wup[:, ds(0, 128)],
                                 wup[:, ds(128, 512)],
                                 start=(i == 0), stop=(i == WARMUP_MMS - 1))

            # startup: the first real matmul needs ansT[k0] + the group-0
            # k0 ent slab.  Those two DMAs are issued in parallel, one on
            # the SP ring (idle until the first score write) and one on
            # the ACT ring, so the dependency lands ~1us earlier than a
            # serial issue on one ring.  Group 0 is deliberately small
            # (2 tiles): the warm loop's first k-cycles then consume input
            # at ~290 GB/s, inside what HBM can deliver while the queues
            # are still ramping.
            ent_sb0 = [ent_pool.tile([128, GN_FULL], bf16, name="ent_sb")
                       for _ in range(KCH)]
            gn0 = GROUPS[0] * NTILE
            nc.sync.dma_start(ansT_sb[:, 0], ansT[ts(0, 128), :])
            nc.scalar.dma_start(ent_sb0[0][:, ds(0, gn0)],
                                entT[ts(0, 128), ds(0, gn0)])
            for k in range(1, KCH):
                nc.scalar.dma_start(ansT_sb[:, k], ansT[ts(k, 128), :])
                nc.scalar.dma_start(ent_sb0[k][:, ds(0, gn0)],
                                    entT[ts(k, 128), ds(0, gn0)])

            # inputs ride the ACT HWDGE ring (nc.scalar) — it keeps
            # prefetches from queueing behind score-output DMAs on SP.
            def load_group(g, gcol):
                # one tile per k-chunk so a matmul only waits for its own DMA
                gn = GROUPS[g] * NTILE
                tiles = []
                for k in range(KCH):
                    t = ent_pool.tile([128, GN_FULL], bf16, name="ent_sb")
                    nc.scalar.dma_start(t[:, ds(0, gn)],
                                        entT[ts(k, 128), ds(gcol, gn)])
                    tiles.append(t)
                return tiles

            # gpsimd (Pool) cannot read PSUM on TRN2 — copyback on DVE + Act
            copy_engines = [nc.vector, nc.scalar]
            ci = 0

            def copyback(dst, ps):
                nonlocal ci
                eng = copy_engines[ci % len(copy_engines)]
                ci += 1
                if eng is nc.scalar:
                    eng.copy(dst, ps)
                else:
                    eng.tensor_copy(out=dst, in_=ps)

            ent_tiles = {0: ent_sb0}
            gcols = np.cumsum([0] + [gs * NTILE for gs in GROUPS]).tolist()
            COLP = gcols[-1]               # partial bf16 tile column base
            COL8 = NB                      # fp8 region column base

            def load_fp8():
                nc.scalar.dma_start(ans8_sb[:], ans8[:, :, :])
                nc.scalar.dma_start(ent8_sb[:, ds(0, 2)], ent8[:, ds(0, 2), :])
                nc.scalar.dma_start(ent8_sb[:, ds(2, 2)], ent8[:, ds(2, 2), :])

            def load_part():
                nc.scalar.dma_start(entP_sb[:], entP[:, :, :])

            def bf16_group(g, warm=False):
                gsz = GROUPS[g]
                gn = gsz * NTILE
                col = gcols[g]
                ent_sb = ent_tiles.pop(g)

                if warm:
                    # warm-up: k-outer with m0..m3 interleaved (4*gsz = 8
                    # psum banks) so each k ent slab feeds 16 matmuls
                    # (~3.5us) — each k-cycle only consumes a 256 KB slab,
                    # so the ramping input queue keeps the PE fed
                    WM = 4
                    outs = [out_pool.tile([128, GN_FULL], f16, name="out_sb")
                            for _ in range(WM)]
                    pss0 = [[psum_pool.tile([128, NTILE], f32, name="pst")
                             for _ in range(gsz)] for _ in range(WM)]
                    for k in range(KCH):
                        for m in range(WM):
                            lhsT = ansT_sb[:, k, ts(m, 128)]
                            for t in range(gsz):
                                nc.tensor.matmul(
                                    pss0[m][t][:], lhsT,
                                    ent_sb[k][:, ts(t, NTILE)],
                                    start=(k == 0), stop=(k == KCH - 1))
                                if k == KCH - 1:
                                    copyback(outs[m][:, ts(t, NTILE)],
                                             pss0[m][t][:])
                    for m in range(WM):
                        nc.sync.dma_start(score[ts(m, 128), ds(col, gn)],
                                          outs[m][:, ds(0, gn)])
                    ms = range(WM, MCH)
                else:
                    ms = range(MCH)

                for m in ms:
                    pss = [psum_pool.tile([128, NTILE], f32, name="pst")
                           for _ in range(gsz)]
                    out_sb = out_pool.tile([128, GN_FULL], f16, name="out_sb")
                    # k outer: keeps the PE streaming one ent tile after
                    # another with the same weight chunk
                    for k in range(KCH):
                        lhsT = ansT_sb[:, k, ts(m, 128)]
                        for t in range(gsz):
                            nc.tensor.matmul(
                                pss[t][:], lhsT, ent_sb[k][:, ts(t, NTILE)],
                                start=(k == 0), stop=(k == KCH - 1))
                    for t in range(gsz):
                        copyback(out_sb[:, ts(t, NTILE)], pss[t][:])
                    if gsz >= 4:
                        # two half-width output DMAs so the drain starts as
                        # soon as the first copies land
                        h0 = (gsz // 2 + 1) * NTILE
                        nc.sync.dma_start(score[ts(m, 128), ds(col, h0)],
                                          out_sb[:, ds(0, h0)])
                        nc.sync.dma_start(
                            score[ts(m, 128), ds(col + h0, gn - h0)],
                            out_sb[:, ds(h0, gn - h0)])
                    else:
                        nc.sync.dma_start(score[ts(m, 128), ds(col, gn)],
                                          out_sb[:, ds(0, gn)])

            def fp8_section():
                # fp8 DoubleRow: K=512 as 2 matmuls of 256 (2 fp8/cell).
                # Runs second to last: its score writes come at 2x the bf16
                # rate (~300 GB/s); the SP ring absorbs them at ~340 GB/s
                # once the input loads are done, and the final small bf16
                # group drains the residue.
                for m in range(MCH):
                    out_sb = out8_pool.tile([128, NF], f16, name="out8_sb")
                    for t in range(NF_FULL):
                        ps = psum_pool.tile([128, NTILE], f32, name="pst")
                        for j in range(2):
                            nc.tensor.matmul(
                                ps[:],
                                ans8_sb[:, ds(2 * j, 2), ts(m, 128)],
                                ent8_sb[:, ds(2 * j, 2), ds(t * NTILE, NTILE)],
                                start=(j == 0), stop=(j == 1),
                                perf_mode=DR)
                        copyback(out_sb[:, ts(t, NTILE)], ps[:])
                    # trailing partial fp8 tile
                    ps = psum_pool.tile([128, NTILE], f32, name="pst")
                    for j in range(2):
                        nc.tensor.matmul(
                            ps[:, ds(0, NF_PART)],
                            ans8_sb[:, ds(2 * j, 2), ts(m, 128)],
                            ent8_sb[:, ds(2 * j, 2),
                                    ds(NF_FULL * NTILE, NF_PART)],
                            start=(j == 0), stop=(j == 1),
                            perf_mode=DR)
                    copyback(out_sb[:, ds(NF_FULL * NTILE, NF_PART)],
                             ps[:, ds(0, NF_PART)])
                    h0 = 7 * NTILE
                    nc.sync.dma_start(score[ts(m, 128), ds(COL8, h0)],
                                      out_sb[:, ds(0, h0)])
                    nc.sync.dma_start(score[ts(m, 128), ds(COL8 + h0, NF - h0)],
                                      out_sb[:, ds(h0, NF - h0)])

            def final_group():
                # final 768-column bf16 group (1 full tile + the 256-col
                # partial): ~11us of PE time that absorbs the fp8 burst's
                # copyback/DMA drain.  Full-tile writes go on SP; the
                # partial's copy+write run on ACT/its own issue so the last
                # m-step's chain is one small 64KB DMA.
                for m in range(MCH):
                    psf = psum_pool.tile([128, NTILE], f32, name="pst")
                    psp = psum_pool.tile([128, NTILE], f32, name="pst")
                    out_sb = out_pool.tile([128, GN_FULL], f16, name="out_sb")
                    for k in range(KCH):
                        lhsT = ansT_sb[:, k, ts(m, 128)]
                        nc.tensor.matmul(psf[:], lhsT,
                                         entP_sb[:, k, ds(0, NTILE)],
                                         start=(k == 0), stop=(k == KCH - 1))
                    for k in range(KCH):
                        lhsT = ansT_sb[:, k, ts(m, 128)]
                        nc.tensor.matmul(psp[:, ds(0, NB_PART)], lhsT,
                                         entP_sb[:, k, ds(NTILE, NB_PART)],
                                         start=(k == 0), stop=(k == KCH - 1))
                    nc.vector.tensor_copy(out=out_sb[:, ds(0, NTILE)],
                                          in_=psf[:])
                    nc.sync.dma_start(score[ts(m, 128), ds(COLP, NTILE)],
                                      out_sb[:, ds(0, NTILE)])
                    nc.scalar.copy(out_sb[:, ds(NTILE, NB_PART)],
                                   psp[:, ds(0, NB_PART)])
                    nc.scalar.dma_start(
                        score[ts(m, 128), ds(COLP + NTILE, NB_PART)],
                        out_sb[:, ds(NTILE, NB_PART)])

            # process order: b0..b5, fp8, final — each section's inputs are
            # issued at least one section ahead on the ACT ring.
            ent_tiles[1] = load_group(1, gcols[1])
            bf16_group(0, warm=True)
            ent_tiles[2] = load_group(2, gcols[2])
            bf16_group(1)
            ent_tiles[3] = load_group(3, gcols[3])
            bf16_group(2)
            ent_tiles[4] = load_group(4, gcols[4])
            bf16_group(3)
            load_fp8()
            bf16_group(4)
            ent_tiles[5] = load_group(5, gcols[5])
            load_part()
            bf16_group(5)
            fp8_section()
            final_group()
            for _f in reversed(_frees):
                _f()
    nc.compile()
    return nc


def _get_nc():
    global _NC
    if _NC is None:
        _NC = _build_nc()
    return _NC


def _pmap(fn, n):
    from concurrent.futures import ThreadPoolExecutor
    with ThreadPoolExecutor(max_workers=n) as ex:
        list(ex.map(fn, range(n)))


def _to_f8_chunks(mat_t, ncols):
    """[EMB, ncols] f32 (already scaled) -> [128, KCH, ncols] e4m3 bytes."""
    q = mat_t.astype(ml_dtypes.float8_e4m3fn)
    return np.ascontiguousarray(q.reshape(KCH, 128, ncols).transpose(1, 0, 2))


def prepare_in_maps(triples, ent_emb, rel_emb):
    triples = np.asarray(triples)
    ent_emb = np.asarray(ent_emb, dtype=np.float32)
    rel_emb = np.asarray(rel_emb, dtype=np.float32)

    d = EMB // 2
    h = ent_emb[triples[:, 0].astype(np.int64)]
    r = rel_emb[triples[:, 1].astype(np.int64)]
    re_h, im_h = h[:, :d], h[:, d:]
    re_r, im_r = r[:, :d], r[:, d:]
    ans = np.empty((B, EMB), np.float32)
    ans[:, :d] = re_h * re_r - im_h * im_r
    ans[:, d:] = re_h * im_r + im_h * re_r

    ansT_bf = np.ascontiguousarray(ans.T * np.float32(OUT_SCALE)).astype(
        ml_dtypes.bfloat16)

    # fp8 noise per column scales with the column's score energy, i.e. with
    # ||ent_row||^2 — so the smallest-norm entities of each shard go to the
    # fp8 region.  The host scatters columns back during assemble.
    norms = (ent_emb * ent_emb).sum(1)
    bf_idx = np.empty((NCORES, NB), np.int64)
    f8_idx = np.empty((NCORES, NF_REAL), np.int64)
    for c in range(NCORES):
        sl = norms[c * SHARD:(c + 1) * SHARD]
        order = np.argpartition(sl, NF_REAL)
        f8_idx[c] = np.sort(order[:NF_REAL])
        bf_idx[c] = np.sort(order[NF_REAL:])
    _SCALES["bf_idx"] = bf_idx
    _SCALES["f8_idx"] = f8_idx

    # fp8 scales: map absmax to ~120 (TRN e4m3 max 240), then cap the product
    # so the Cauchy-Schwarz bound on device-side scores stays inside fp16
    f8_rows = np.concatenate([
        ent_emb[c * SHARD + f8_idx[c]] for c in range(NCORES)])
    amax_a = float(np.abs(ans).max())
    amax_e = float(np.abs(f8_rows).max())
    s_a = 120.0 / amax_a
    s_e = 120.0 / amax_e
    cs = float(np.sqrt((ans * ans).sum(1).max()) *
               np.sqrt((f8_rows * f8_rows).sum(1).max()))
    cap = 58000.0 / cs
    if s_a * s_e > cap:
        s_a = cap / s_e
    _SCALES["fp8_inv"] = 1.0 / (s_a * s_e)

    ans8 = _to_f8_chunks(np.ascontiguousarray(ans.T) * np.float32(s_a), B)

    ent_bf = np.empty((NCORES, EMB, NGRP), dtype=ml_dtypes.bfloat16)
    ent_pp = np.empty((NCORES, 128, KCH, NFIN), dtype=ml_dtypes.bfloat16)
    ent8s = np.empty((NCORES, 128, KCH, NF), dtype=ml_dtypes.float8_e4m3fn)

    def _core(c):
        rows = ent_emb[c * SHARD:(c + 1) * SHARD]
        bfT = rows[bf_idx[c]].T                      # [EMB, NB]
        ent_bf[c] = bfT[:, :NGRP]
        pp = np.ascontiguousarray(bfT[:, NGRP:])
        ent_pp[c] = pp.reshape(KCH, 128, NFIN).transpose(1, 0, 2)
        blk = np.zeros((EMB, NF), np.float32)
        blk[:, :NF_REAL] = rows[f8_idx[c]].T * np.float32(s_e)
        ent8s[c] = _to_f8_chunks(blk, NF)

    _pmap(_core, NCORES)
    return [{"ansT": ansT_bf, "ans8": ans8, "entT": ent_bf[c],
             "entP": ent_pp[c], "ent8": ent8s[c]} for c in range(NCORES)]


def run_raw(in_maps, trace=False):
    from concourse import bass_utils
    return bass_utils.run_bass_kernel_spmd(
        _get_nc(), in_maps, core_ids=list(range(NCORES)), trace=trace
    )


def assemble(results):
    out = np.empty((B, NUM_ENT), np.float32)
    inv16 = np.float32(1.0 / OUT_SCALE)
    inv8 = np.float32(_SCALES["fp8_inv"])
    bf_idx = _SCALES["bf_idx"]
    f8_idx = _SCALES["f8_idx"]

    def _one(c):
        sh = results[c]["score"]
        bf = sh[:, :NB].astype(np.float32)
        bf *= inv16
        f8 = sh[:, NB:NB + NF_REAL].astype(np.float32)
        f8 *= inv8
        base = c * SHARD
        out[:, base + bf_idx[c]] = bf
        out[:, base + f8_idx[c]] = f8

    _pmap(_one, NCORES)
    return out


def kernel(triples, ent_emb, rel_emb):
    in_maps = prepare_in_maps(triples, ent_emb, rel_emb)
    res = run_raw(in_maps)
    return assemble(res.results)


# revision 11
# speedup vs baseline: 1.0267x; 1.0017x over previous
"""ComplEx scoring kernel for 8 Trainium2 NeuronCores.

Math: score[b, e] = Re(<h_b * r_b, conj(ent_e)>) with h = ent_emb[triples[:,0]],
r = rel_emb[triples[:,1]].  Writing ans_b = concat(re_h*re_r - im_h*im_r,
re_h*im_r + im_h*re_r) (shape [B, 512]), the score is exactly
score = ans @ ent_emb.T  — one [1024, 512] x [512, 200000] GEMM.

Strategy (vocab/tensor parallel on the entity axis, 25000 entities/core,
padded to 25088 = 49x512 columns):
  - host: tiny gather + complex multiply -> ans  (microseconds)
  - the GEMM is TensorE-bound (26.3 GFLOP/core vs 78.6 TF/s bf16), so the
    entity axis is split into a bf16 part (34.5 tiles of 512) and an
    fp8-e4m3 DoubleRow part (14.5 tiles) running the PE at 2 MACs/cell/cy.
  - fp8 noise is relative per column, so each column's squared error is
    proportional to its score energy ~ ||ent_row||^2.  Assigning the
    smallest-norm entities of each shard to the fp8 region (instead of an
    arbitrary 26%) cuts the fp8 error ~8%, which funds a 29.3% fp8
    fraction at the same global rel err (~1.96e-2 < 2e-2).  The host
    unscrambles the column permutation during assemble.
  - per core: score_bf16[1024, 17664] + score_fp8[1024, 7424], both f16.
    PE is pre-warmed with dummy matmuls so the HAM clock gate opens during
    the preamble/first DMAs.  The first two input DMAs are split across
    the SP and ACT HWDGE rings so the first slab lands ~1us earlier.
    The fp8 section runs second to last; a final 256-column bf16 group
    absorbs its write backlog so the kernel tail is one small DMA.
  - host: per-region unscale + column scatter back to entity order.
"""

import numpy as np
import ml_dtypes

NCORES = 8
NUM_ENT = 200000
EMB = 512
B = 1024
SHARD = NUM_ENT // NCORES      # 25000 entities per core
NTILE = 512                    # matmul moving free dim == one PSUM bank
NB_FULL = 34                   # full bf16 512-tiles per core
NB_PART = 256                  # trailing partial bf16 tile width
NB = NB_FULL * NTILE + NB_PART # 17664 bf16 columns (all real)
NF_FULL = 14                   # full fp8 512-tiles per core
NF_PART = 256                  # trailing partial fp8 tile width
NF = NF_FULL * NTILE + NF_PART # 7424 fp8 columns (7336 real + 88 pad)
SHARD_PAD = NB + NF            # 25088
NF_REAL = SHARD - NB           # 7336 real entities in the fp8 region
GROUPS = [2, 5, 7, 7, 7, 5]    # full bf16 tile groups (DMA/reuse granularity)
NGRP = 33 * NTILE              # columns covered by GROUPS
NFIN = NB - NGRP               # 768: final group = 1 full tile + partial
GN_FULL = 7 * NTILE            # 3584
KCH = EMB // 128               # 4 contraction chunks
MCH = B // 128                 # 8 batch chunks
WARMUP_MMS = 9

_NC = None
_SCALES = {}

# bf16-path score values are ~1e-5 — subnormal in fp16.  Pre-scaling ans by
# 2**16 on the host puts the device-side scores in fp16's normal range; the
# host unscales.  The fp8 path has its own scales (s_a, s_e) chosen at prep
# time so quantized inputs sit in e4m3's range and scores fit fp16.
OUT_SCALE = 2.0 ** 16


def _build_nc():
    import concourse.bacc as bacc
    import concourse.bass as bass
    import concourse.tile as tile
    from concourse import mybir

    ts, ds = bass.ts, bass.ds
    bf16 = mybir.dt.bfloat16
    f16 = mybir.dt.float16
    f8 = mybir.dt.float8e4
    f32 = mybir.dt.float32
    DR = mybir.MatmulPerfMode.DoubleRow

    nc = bacc.Bacc("TRN2", target_bir_lowering=False, debug=False)
    ansT = nc.dram_tensor("ansT", [EMB, B], bf16, kind="ExternalInput")
    ans8 = nc.dram_tensor("ans8", [128, KCH, B], f8, kind="ExternalInput")
    entT = nc.dram_tensor("entT", [EMB, NGRP], bf16, kind="ExternalInput")
    entP = nc.dram_tensor("entP", [128, KCH, NFIN], bf16,
                          kind="ExternalInput")
    ent8 = nc.dram_tensor("ent8", [128, KCH, NF], f8, kind="ExternalInput")
    score = nc.dram_tensor("score", [B, SHARD_PAD], f16, kind="ExternalOutput")

    with tile.TileContext(nc) as tc:
        with tc.tile_pool(name="entp", bufs=2 * KCH) as ent_pool, \
             tc.tile_pool(name="outp", bufs=4) as out_pool, \
             tc.tile_pool(name="out8p", bufs=3) as out8_pool, \
             tc.tile_pool(name="ps", bufs=8, space="PSUM") as psum_pool:

            _frees = []
            ansT_sb, _f = tc.tile([128, KCH, B], bf16, name="ansT_sb")
            _frees.append(_f)
            ans8_sb, _f = tc.tile([128, KCH, B], f8, name="ans8_sb")
            _frees.append(_f)
            entP_sb, _f = tc.tile([128, KCH, NFIN], bf16, name="entP_sb")
            _frees.append(_f)
            ent8_sb, _f = tc.tile([128, KCH, NF], f8, name="ent8_sb")
            _frees.append(_f)
            wup, _f = tc.tile([128, 640], bf16, name="wup")
            _frees.append(_f)

            # PE pre-warm: zero matmuls so the HAM clock gate opens during
            # the preamble/DMA wait; real matmuls then run at 2.4 GHz.
            nc.gpsimd.memset(wup[:], 0)
            wps = psum_pool.tile([128, NTILE], f32, name="pst")
            for i in range(WARMUP_MMS):
                nc.tensor.matmul(wps[:], wup[:, ds(0, 128)],
                                 wup[:, ds(128, 512)],
                                 start=(i == 0), stop=(i == WARMUP_MMS - 1))

            # startup: the first real matmul needs ansT[k0] + the group-0
            # k0 ent slab.  Those two DMAs are issued in parallel, one on
            # the SP ring (idle until the first score write) and one on
            # the ACT ring, so the dependency lands ~1us earlier than a
            # serial issue on one ring.  Group 0 is deliberately small
            # (2 tiles): the warm loop's first k-cycles then consume input
            # at ~290 GB/s, inside what HBM can deliver while the queues
            # are still ramping.
            ent_sb0 = [ent_pool.tile([128, GN_FULL], bf16, name="ent_sb")
                       for _ in range(KCH)]
            gn0 = GROUPS[0] * NTILE
            nc.sync.dma_start(ansT_sb[:, 0], ansT[ts(0, 128), :])
            nc.scalar.dma_start(ent_sb0[0][:, ds(0, gn0)],
                                entT[ts(0, 128), ds(0, gn0)])
            for k in range(1, KCH):
                nc.scalar.dma_start(ansT_sb[:, k], ansT[ts(k, 128), :])
                nc.scalar.dma_start(ent_sb0[k][:, ds(0, gn0)],
                                    entT[ts(k, 128), ds(0, gn0)])

            # inputs ride the ACT HWDGE ring (nc.scalar) — it keeps
            # prefetches from queueing behind score-output DMAs on SP.
            def load_group(g, gcol):
                # one tile per k-chunk so a matmul only waits for its own DMA
                gn = GROUPS[g] * NTILE
                tiles = []
                for k in range(KCH):
                    t = ent_pool.tile([128, GN_FULL], bf16, name="ent_sb")
                    nc.scalar.dma_start(t[:, ds(0, gn)],
                                        entT[ts(k, 128), ds(gcol, gn)])
                    tiles.append(t)
                return tiles

            # gpsimd (Pool) cannot read PSUM on TRN2 — copyback on DVE + Act
            copy_engines = [nc.vector, nc.scalar]
            ci = 0

            def copyback(dst, ps):
                nonlocal ci
                eng = copy_engines[ci % len(copy_engines)]
                ci += 1
                if eng is nc.scalar:
                    eng.copy(dst, ps)
                else:
                    eng.tensor_copy(out=dst, in_=ps)

            ent_tiles = {0: ent_sb0}
            gcols = np.cumsum([0] + [gs * NTILE for gs in GROUPS]).tolist()
            COLP = gcols[-1]               # partial bf16 tile column base
            COL8 = NB                      # fp8 region column base

            def load_fp8():
                nc.scalar.dma_start(ans8_sb[:], ans8[:, :, :])
                nc.scalar.dma_start(ent8_sb[:, ds(0, 2)], ent8[:, ds(0, 2), :])
                nc.scalar.dma_start(ent8_sb[:, ds(2, 2)], ent8[:, ds(2, 2), :])

            def load_part():
                nc.scalar.dma_start(entP_sb[:], entP[:, :, :])

            def bf16_group(g, warm=False):
                gsz = GROUPS[g]
                gn = gsz * NTILE
                col = gcols[g]
                ent_sb = ent_tiles.pop(g)

                if warm:
                    # warm-up: k-outer with m0..m3 interleaved (4*gsz = 8
                    # psum banks) so each k ent slab feeds 16 matmuls
                    # (~3.5us) — each k-cycle only consumes a 256 KB slab,
                    # so the ramping input queue keeps the PE fed
                    WM = 4
                    outs = [out_pool.tile([128, GN_FULL], f16, name="out_sb")
                            for _ in range(WM)]
                    pss0 = [[psum_pool.tile([128, NTILE], f32, name="pst")
                             for _ in range(gsz)] for _ in range(WM)]
                    for k in range(KCH):
                        for m in range(WM):
                            lhsT = ansT_sb[:, k, ts(m, 128)]
                            for t in range(gsz):
                                nc.tensor.matmul(
                                    pss0[m][t][:], lhsT,
                                    ent_sb[k][:, ts(t, NTILE)],
                                    start=(k == 0), stop=(k == KCH - 1))
                                if k == KCH - 1:
                                    copyback(outs[m][:, ts(t, NTILE)],
                                             pss0[m][t][:])
                    for m in range(WM):
                        nc.sync.dma_start(score[ts(m, 128), ds(col, gn)],
                                          outs[m][:, ds(0, gn)])
                    ms = range(WM, MCH)
                else:
                    ms = range(MCH)

                for m in ms:
                    pss = [psum_pool.tile([128, NTILE], f32, name="pst")
                           for _ in range(gsz)]
                    out_sb = out_pool.tile([128, GN_FULL], f16, name="out_sb")
                    # tile outer: each tile's copyback fires right after its
                    # 4 accumulating matmuls, so PSUM banks are freed ~5us
                    # before the pool reuses them (k-outer bunched all the
                    # copies at the m-step's end and start-MMs stalled on
                    # bank recycling every ~49 MMs)
                    for t in range(gsz):
                        for k in range(KCH):
                            nc.tensor.matmul(
                                pss[t][:], ansT_sb[:, k, ts(m, 128)],
                                ent_sb[k][:, ts(t, NTILE)],
                                start=(k == 0), stop=(k == KCH - 1))
                        copyback(out_sb[:, ts(t, NTILE)], pss[t][:])
                    if gsz >= 4:
                        # two half-width output DMAs so the drain starts as
                        # soon as the first copies land
                        h0 = (gsz // 2 + 1) * NTILE
                        nc.sync.dma_start(score[ts(m, 128), ds(col, h0)],
                                          out_sb[:, ds(0, h0)])
                        nc.sync.dma_start(
                            score[ts(m, 128), ds(col + h0, gn - h0)],
                            out_sb[:, ds(h0, gn - h0)])
                    else:
                        nc.sync.dma_start(score[ts(m, 128), ds(col, gn)],
                                          out_sb[:, ds(0, gn)])

            def fp8_section():
                # fp8 DoubleRow: K=512 as 2 matmuls of 256 (2 fp8/cell).
                # Runs second to last: its score writes come at 2x the bf16
                # rate (~300 GB/s); the SP ring absorbs them at ~340 GB/s
                # once the input loads are done, and the final small bf16
                # group drains the residue.
                for m in range(MCH):
                    out_sb = out8_pool.tile([128, NF], f16, name="out8_sb")
                    for t in range(NF_FULL):
                        ps = psum_pool.tile([128, NTILE], f32, name="pst")
                        for j in range(2):
                            nc.tensor.matmul(
                                ps[:],
                                ans8_sb[:, ds(2 * j, 2), ts(m, 128)],
                                ent8_sb[:, ds(2 * j, 2), ds(t * NTILE, NTILE)],
                                start=(j == 0), stop=(j == 1),
                                perf_mode=DR)
                        copyback(out_sb[:, ts(t, NTILE)], ps[:])
                    # trailing partial fp8 tile
                    ps = psum_pool.tile([128, NTILE], f32, name="pst")
                    for j in range(2):
                        nc.tensor.matmul(
                            ps[:, ds(0, NF_PART)],
                            ans8_sb[:, ds(2 * j, 2), ts(m, 128)],
                            ent8_sb[:, ds(2 * j, 2),
                                    ds(NF_FULL * NTILE, NF_PART)],
                            start=(j == 0), stop=(j == 1),
                            perf_mode=DR)
                    copyback(out_sb[:, ds(NF_FULL * NTILE, NF_PART)],
                             ps[:, ds(0, NF_PART)])
                    h0 = 7 * NTILE
                    nc.sync.dma_start(score[ts(m, 128), ds(COL8, h0)],
                                      out_sb[:, ds(0, h0)])
                    nc.sync.dma_start(score[ts(m, 128), ds(COL8 + h0, NF - h0)],
                                      out_sb[:, ds(h0, NF - h0)])

            def final_group():
                # final 768-column bf16 group (1 full tile + the 256-col
                # partial): ~11us of PE time that absorbs the fp8 burst's
                # copyback/DMA drain.  Full-tile writes go on SP; the
                # partial's copy+write run on ACT/its own issue so the last
                # m-step's chain is one small 64KB DMA.
                for m in range(MCH):
                    psf = psum_pool.tile([128, NTILE], f32, name="pst")
                    psp = psum_pool.tile([128, NTILE], f32, name="pst")
                    out_sb = out_pool.tile([128, GN_FULL], f16, name="out_sb")
                    for k in range(KCH):
                        lhsT = ansT_sb[:, k, ts(m, 128)]
                        nc.tensor.matmul(psf[:], lhsT,
                                         entP_sb[:, k, ds(0, NTILE)],
                                         start=(k == 0), stop=(k == KCH - 1))
                    for k in range(KCH):
                        lhsT = ansT_sb[:, k, ts(m, 128)]
                        nc.tensor.matmul(psp[:, ds(0, NB_PART)], lhsT,
                                         entP_sb[:, k, ds(NTILE, NB_PART)],
                                         start=(k == 0), stop=(k == KCH - 1))
                    nc.vector.tensor_copy(out=out_sb[:, ds(0, NTILE)],
                                          in_=psf[:])
                    nc.sync.dma_start(score[ts(m, 128), ds(COLP, NTILE)],
                                      out_sb[:, ds(0, NTILE)])
                    nc.scalar.copy(out_sb[:, ds(NTILE, NB_PART)],
                                   psp[:, ds(0, NB_PART)])
                    nc.scalar.dma_start(
                        score[ts(m, 128), ds(COLP + NTILE, NB_PART)],
                        out_sb[:, ds(NTILE, NB_PART)])

            # process order: b0..b5, fp8, final — each section's inputs are
            # issued at least one section ahead on the ACT ring.
            ent_tiles[1] = load_group(1, gcols[1])
            bf16_group(0, warm=True)
            ent_tiles[2] = load_group(2, gcols[2])
            bf16_group(1)
            ent_tiles[3] = load_group(3, gcols[3])
            bf16_group(2)
            ent_tiles[4] = load_group(4, gcols[4])
            bf16_group(3)
            load_fp8()
            bf16_group(4)
            ent_tiles[5] = load_group(5, gcols[5])
            load_part()
            bf16_group(5)
            fp8_section()
            final_group()
            for _f in reversed(_frees):
                _f()
    nc.compile()
    return nc


def _get_nc():
    global _NC
    if _NC is None:
        _NC = _build_nc()
    return _NC


def _pmap(fn, n):
    from concurrent.futures import ThreadPoolExecutor
    with ThreadPoolExecutor(max_workers=n) as ex:
        list(ex.map(fn, range(n)))


def _to_f8_chunks(mat_t, ncols):
    """[EMB, ncols] f32 (already scaled) -> [128, KCH, ncols] e4m3 bytes."""
    q = mat_t.astype(ml_dtypes.float8_e4m3fn)
    return np.ascontiguousarray(q.reshape(KCH, 128, ncols).transpose(1, 0, 2))


def prepare_in_maps(triples, ent_emb, rel_emb):
    triples = np.asarray(triples)
    ent_emb = np.asarray(ent_emb, dtype=np.float32)
    rel_emb = np.asarray(rel_emb, dtype=np.float32)

    d = EMB // 2
    h = ent_emb[triples[:, 0].astype(np.int64)]
    r = rel_emb[triples[:, 1].astype(np.int64)]
    re_h, im_h = h[:, :d], h[:, d:]
    re_r, im_r = r[:, :d], r[:, d:]
    ans = np.empty((B, EMB), np.float32)
    ans[:, :d] = re_h * re_r - im_h * im_r
    ans[:, d:] = re_h * im_r + im_h * re_r

    ansT_bf = np.ascontiguousarray(ans.T * np.float32(OUT_SCALE)).astype(
        ml_dtypes.bfloat16)

    # fp8 noise per column scales with the column's score energy, i.e. with
    # ||ent_row||^2 — so the smallest-norm entities of each shard go to the
    # fp8 region.  The host scatters columns back during assemble.
    norms = (ent_emb * ent_emb).sum(1)
    bf_idx = np.empty((NCORES, NB), np.int64)
    f8_idx = np.empty((NCORES, NF_REAL), np.int64)
    for c in range(NCORES):
        sl = norms[c * SHARD:(c + 1) * SHARD]
        order = np.argpartition(sl, NF_REAL)
        f8_idx[c] = np.sort(order[:NF_REAL])
        bf_idx[c] = np.sort(order[NF_REAL:])
    _SCALES["bf_idx"] = bf_idx
    _SCALES["f8_idx"] = f8_idx

    # fp8 scales: map absmax to ~120 (TRN e4m3 max 240), then cap the product
    # so the Cauchy-Schwarz bound on device-side scores stays inside fp16
    f8_rows = np.concatenate([
        ent_emb[c * SHARD + f8_idx[c]] for c in range(NCORES)])
    amax_a = float(np.abs(ans).max())
    amax_e = float(np.abs(f8_rows).max())
    s_a = 120.0 / amax_a
    s_e = 120.0 / amax_e
    cs = float(np.sqrt((ans * ans).sum(1).max()) *
               np.sqrt((f8_rows * f8_rows).sum(1).max()))
    cap = 58000.0 / cs
    if s_a * s_e > cap:
        s_a = cap / s_e
    _SCALES["fp8_inv"] = 1.0 / (s_a * s_e)

    ans8 = _to_f8_chunks(np.ascontiguousarray(ans.T) * np.float32(s_a), B)

    ent_bf = np.empty((NCORES, EMB, NGRP), dtype=ml_dtypes.bfloat16)
    ent_pp = np.empty((NCORES, 128, KCH, NFIN), dtype=ml_dtypes.bfloat16)
    ent8s = np.empty((NCORES, 128, KCH, NF), dtype=ml_dtypes.float8_e4m3fn)

    def _core(c):
        rows = ent_emb[c * SHARD:(c + 1) * SHARD]
        bfT = rows[bf_idx[c]].T                      # [EMB, NB]
        ent_bf[c] = bfT[:, :NGRP]
        pp = np.ascontiguousarray(bfT[:, NGRP:])
        ent_pp[c] = pp.reshape(KCH, 128, NFIN).transpose(1, 0, 2)
        blk = np.zeros((EMB, NF), np.float32)
        blk[:, :NF_REAL] = rows[f8_idx[c]].T * np.float32(s_e)
        ent8s[c] = _to_f8_chunks(blk, NF)

    _pmap(_core, NCORES)
    return [{"ansT": ansT_bf, "ans8": ans8, "entT": ent_bf[c],
             "entP": ent_pp[c], "ent8": ent8s[c]} for c in range(NCORES)]


def run_raw(in_maps, trace=False):
    from concourse import bass_utils
    return bass_utils.run_bass_kernel_spmd(
        _get_nc(), in_maps, core_ids=list(range(NCORES)), trace=trace
    )


def assemble(results):
    out = np.empty((B, NUM_ENT), np.float32)
    inv16 = np.float32(1.0 / OUT_SCALE)
    inv8 = np.float32(_SCALES["fp8_inv"])
    bf_idx = _SCALES["bf_idx"]
    f8_idx = _SCALES["f8_idx"]

    def _one(c):
        sh = results[c]["score"]
        bf = sh[:, :NB].astype(np.float32)
        bf *= inv16
        f8 = sh[:, NB:NB + NF_REAL].astype(np.float32)
        f8 *= inv8
        base = c * SHARD
        out[:, base + bf_idx[c]] = bf
        out[:, base + f8_idx[c]] = f8

    _pmap(_one, NCORES)
    return out


def kernel(triples, ent_emb, rel_emb):
    in_maps = prepare_in_maps(triples, ent_emb, rel_emb)
    res = run_raw(in_maps)
    return assemble(res.results)
